# revision 8
# baseline (speedup 1.0000x reference)
"""3-layer GCN node classifier on 8 Trainium2 NeuronCores.

Math (per layer, folding the symmetric normalization):
    deg[v]  = in-degree(v) (with self loop), dinv = rsqrt(deg)
    g       = dinv * (h @ W)                  (rows scaled)
    out[c]  = dinv[c] * ( sum_{e: col=c} g[row_e] + g[c] ) + b
    h_next  = relu(out)      (layers 1,2; layer 3 has no relu)

Distribution: nodes are range-sharded across 8 cores (graph parallel).
Each core computes g for its own nodes (dense matmul), the g-table is
all-gathered to every core's HBM, each core then gathers the rows for
the edges whose *destination* it owns (SWDGE dma_gather) and
scatter-adds them into per-destination-tile PSUM accumulators via
one-hot matmuls on the TensorEngine.

Perf notes vs the original baseline:
  * dma_gather cost is ~per-descriptor (bytes nearly free) and a single
    SWDGE queue saturates at ~9 ns/desc; spreading calls across 4 queues
    reaches ~3 ns/desc. All gathers round-robin over queue_num 0..3.
  * layers 1-2 use an fp16 table/messages/one-hot/weights (256B gather
    rows, 2x PE rate, halved AllGather); layer 3 keeps fp32 (64ch * 2B
    = 128B would violate the 256B gather row-size constraint).
  * one-hot tiles are generated in one batched DVE op per destination
    tile (broadcast APs), fp16 output for the 2x 16-bit DVE rate.

Host-side preprocessing only touches edge_index (graph structure):
CSR-style bucketing of edges by destination tile, degree computation,
a within-core node permutation that load-balances destination tiles,
and int16 gather-index packing (the 50k-row table is split into two
overlapping <=32768-row views because SWDGE gather indices are int16).
"""

import math
import os
import numpy as np

# ---------------------------------------------------------------------------
# problem constants (hardcoded per contract; kernel.py must be self-contained)
# ---------------------------------------------------------------------------
N_NODES = 50000
IN_C, HID_C, OUT_C = 128, 128, 64
M_CORES = 8
NPC = N_NODES // M_CORES            # 6250 nodes per core
TPC = (NPC + 127) // 128            # 49 destination tiles per core
NPAD = TPC * 128                    # 6272 padded nodes per core
TBL = M_CORES * NPAD                # 50176 rows in the all-gathered table
LO_CAP = 32768                      # int16 index reach
HI_OFF = max(0, TBL - 32768)        # 17408: hi view = table[HI_OFF:]
NQ = 4                              # SWDGE queues used round-robin

F32 = "float32"


# ---------------------------------------------------------------------------
# host-side graph preprocessing (indices only)
# ---------------------------------------------------------------------------
def _plan(edge_index: np.ndarray):
    """Build per-core index/metadata arrays from edge_index [2, E]."""
    row = np.asarray(edge_index[0], dtype=np.int64)
    col = np.asarray(edge_index[1], dtype=np.int64)

    deg_in = np.bincount(col, minlength=N_NODES)          # edges only
    dinv = 1.0 / np.sqrt(deg_in + 1.0)                     # + self loop

    # within-core permutation: snake-fill tiles with degree-sorted nodes so
    # every destination tile carries a near-equal number of incoming edges.
    pos_local = np.empty(N_NODES, dtype=np.int64)
    for k in range(M_CORES):
        v0 = k * NPC
        d = deg_in[v0 : v0 + NPC]
        order = np.argsort(-d, kind="stable")              # heavy first
        # serpentine tile ids: 0..T-1, T-1..0, ...
        idx = np.arange(NPC)
        rnd, off = divmod(idx, TPC)
        tile_ids = np.where(rnd % 2 == 0, off, TPC - 1 - off)
        slot_in_tile = rnd
        pos = tile_ids * 128 + slot_in_tile
        pos_local[v0 + order] = pos

    g_pos = (np.arange(N_NODES) // NPC) * NPAD + pos_local  # table row per node

    # per-core per-tile edge buckets
    kd = col // NPC
    src_pos = g_pos[row]
    dst_slot = pos_local[col]
    dst_tile = dst_slot // 128
    dst_loc = dst_slot % 128
    is_lo = src_pos < LO_CAP

    # counts to size K_lo / K_hi uniformly across the SPMD program
    tile_key = kd * TPC + dst_tile
    n_lo = np.bincount(tile_key[is_lo], minlength=M_CORES * TPC)
    n_hi = np.bincount(tile_key[~is_lo], minlength=M_CORES * TPC)
    K_lo = max(1, int(math.ceil(n_lo.max() / 128)))
    K_hi = max(1, int(math.ceil(n_hi.max() / 128)))
    K = K_lo + K_hi
    NCH = TPC * K

    per_core = []
    for k in range(M_CORES):
        idx16 = np.zeros((TPC, K, 128), dtype=np.int16)    # pad -> row 0
        dloc_a = np.full((TPC, K, 128), 200.0, dtype=np.float32)
        m = kd == k
        tl, lc, sp, lo = dst_tile[m], dst_loc[m], src_pos[m], is_lo[m]
        for t in range(TPC):
            mt = tl == t
            for stream, base in ((lo & mt, 0), ((~lo) & mt, K_lo)):
                sps = sp[stream]
                lcs = lc[stream]
                n = sps.size
                if base == 0:
                    vals = sps
                else:
                    vals = sps - HI_OFF
                flat_i = idx16[t].reshape(-1)
                flat_d = dloc_a[t].reshape(-1)
                o = base * 128
                flat_i[o : o + n] = vals.astype(np.int16)
                flat_d[o : o + n] = lcs.astype(np.float32)

        # SWDGE wrapped-16 index layout, replicated across the 8 Q7 groups
        flat = idx16.reshape(-1)
        wrapped = flat.reshape(-1, 16).T                    # [16, NCH*8]
        idx_sb = np.tile(wrapped, (8, 1)).copy()            # [128, NCH*8]
        # per-chunk destination-local column, lane-major
        dloc_sb = dloc_a.transpose(2, 0, 1).reshape(128, NCH).copy()
        # per-slot dinv (0 on dummy slots)
        dv = np.zeros(NPAD, dtype=np.float32)
        v0 = k * NPC
        dv[pos_local[v0 : v0 + NPC]] = dinv[v0 : v0 + NPC]
        dinv_sb = dv.reshape(TPC, 128).T.copy()             # [128, TPC]
        per_core.append(dict(idx=idx_sb, dloc=dloc_sb, dinv=dinv_sb))

    return dict(
        K_lo=K_lo, K_hi=K_hi, NCH=NCH, per_core=per_core,
        pos_local=pos_local, dinv=dinv,
    )


# ---------------------------------------------------------------------------
# device program
# ---------------------------------------------------------------------------
def _build_nc(K_lo: int, K_hi: int, with_bias: bool, reps: int = 0, ablate: str = ''):
    abl = set(a for a in ablate.split(',') if a)
    """Build + compile the SPMD program.

    reps > 0 additionally emits a timing loop: the full pipeline runs once
    (correct, fills the gful tables), then a hardware For_i loop re-runs
    the whole body `reps` times with the collectives elided (collectives
    cannot sit inside control flow) so device time dominates wall clock.
    """
    import concourse.bacc as bacc
    import concourse.mybir as mybir
    from concourse import tile
    from concourse._compat import get_trn_type

    dt = mybir.dt
    K = K_lo + K_hi
    NCH = TPC * K
    NW = NCH * 8
    F16 = dt.float16

    nc = bacc.Bacc(
        get_trn_type() or "TRN2",
        target_bir_lowering=False,
        debug=False,
        enable_asserts=False,
        num_devices=M_CORES,
        num_swdge_queues=NQ,
    )

    # I/O
    xT_p = nc.dram_tensor("xT", [128, NPAD], F16, kind="ExternalInput")
    W1_p = nc.dram_tensor("W1", [IN_C, HID_C], F16, kind="ExternalInput")
    W2_p = nc.dram_tensor("W2", [HID_C, HID_C], F16, kind="ExternalInput")
    W3_p = nc.dram_tensor("W3", [HID_C, OUT_C], F16, kind="ExternalInput")
    dinv_p = nc.dram_tensor("dinv", [128, TPC], dt.float32, kind="ExternalInput")
    dloc_p = nc.dram_tensor("dloc", [128, NCH], F16, kind="ExternalInput")
    idx_p = nc.dram_tensor("idx", [128, NW], dt.int16, kind="ExternalInput")
    iota_p = nc.dram_tensor("iota", [128, 128], F16, kind="ExternalInput")
    ident_p = nc.dram_tensor("ident", [128, 128], dt.float32, kind="ExternalInput")
    if with_bias:
        b1_p = nc.dram_tensor("b1r", [128, HID_C], dt.float32, kind="ExternalInput")
        b2_p = nc.dram_tensor("b2r", [128, HID_C], dt.float32, kind="ExternalInput")
        b3_p = nc.dram_tensor("b3r", [128, OUT_C], dt.float32, kind="ExternalInput")
    out_p = nc.dram_tensor("out", [NPAD, OUT_C], dt.float32, kind="ExternalOutput")

    RG = [list(range(M_CORES))]
    AF = mybir.ActivationFunctionType
    OP = mybir.AluOpType

    with tile.TileContext(nc) as tc, tc.tile_pool(name="persist", bufs=1) as pp:
        # persistent SBUF tiles (one slot each)
        hT_a = pp.tile([128, NPAD], F16, name="hT_a")
        hT_b = pp.tile([128, NPAD], F16, name="hT_b")
        w1_sb = pp.tile([128, HID_C], F16, name="w1_sb")
        w2_sb = pp.tile([128, HID_C], F16, name="w2_sb")
        w3_sb = pp.tile([128, OUT_C], F16, name="w3_sb")
        dinv_sb = pp.tile([128, TPC], dt.float32, name="dinv_sb")
        dloc_sb = pp.tile([128, NCH], F16, name="dloc_sb")
        idx_sb = pp.tile([128, NW], dt.int16, name="idx_sb")
        iota_sb = pp.tile([128, 128], F16, name="iota_sb")
        ident_sb = pp.tile([128, 128], dt.float32, name="ident_sb")
        bias_sb = []

        nc.sync.dma_start(hT_a[:], xT_p[:])
        nc.sync.dma_start(w1_sb[:], W1_p[:])
        nc.sync.dma_start(w2_sb[:], W2_p[:])
        nc.sync.dma_start(w3_sb[:], W3_p[:])
        nc.sync.dma_start(dinv_sb[:], dinv_p[:])
        nc.sync.dma_start(dloc_sb[:], dloc_p[:])
        nc.sync.dma_start(idx_sb[:], idx_p[:])
        nc.sync.dma_start(iota_sb[:], iota_p[:])
        nc.sync.dma_start(ident_sb[:], ident_p[:])
        if with_bias:
            for p, cc in ((b1_p, HID_C), (b2_p, HID_C), (b3_p, OUT_C)):
                t = pp.tile([128, cc], dt.float32, name=f"bias{len(bias_sb)}_sb")
                nc.sync.dma_start(t[:], p[:])
                bias_sb.append(t)

        # (w_sb, C, relu, table dtype, hT_in, hT_out)
        layers = [
            (w1_sb, HID_C, True, F16, hT_a, hT_b),
            (w2_sb, HID_C, True, F16, hT_b, hT_a),
            (w3_sb, OUT_C, False, dt.float32, hT_a, None),
        ]

        with (
            tc.tile_pool(name="gsb", bufs=2) as gsb_pool,
            tc.tile_pool(name="msg", bufs=2 * NQ) as msg_pool,
            tc.tile_pool(name="oh", bufs=4) as oh_pool,
            tc.tile_pool(name="eps", bufs=3) as eps_pool,
            tc.tile_pool(name="psA", bufs=2, space="PSUM") as psA_pool,
            tc.tile_pool(name="psS", bufs=2, space="PSUM") as psS_pool,
            tc.tile_pool(name="psT", bufs=2, space="PSUM") as psT_pool,
            tc.tile_pool(name="dram", bufs=1, space="DRAM") as dram_pool,
        ):
            glocs = [
                dram_pool.tile([NPAD, c], d, name=f"gloc{i}")
                for i, (c, d) in enumerate(
                    [(HID_C, F16), (HID_C, F16), (OUT_C, dt.float32)]
                )
            ]
            gfuls = [
                dram_pool.tile(
                    [TBL, c], d,
                    addr_space="Shared" if M_CORES > 4 else "Local",
                    name=f"gful{i}",
                )
                for i, (c, d) in enumerate(
                    [(HID_C, F16), (HID_C, F16), (OUT_C, dt.float32)]
                )
            ]

            qload = [0] * NQ

            def next_q(ndesc):
                q = min(range(NQ), key=lambda i: qload[i])
                qload[q] += ndesc
                return q

            def emit_layers(with_cc):
                for li, (w_sb, C, relu, tdt, hT_in, hT_out) in enumerate(layers):
                    gloc, gful = glocs[li], gfuls[li]

                    # stage A: g = dinv * (h @ W) for own nodes
                    g_sb = gsb_pool.tile([128, TPC, C], tdt, tag=f"gsb{li}")
                    for t in range(TPC):
                        psA = psA_pool.tile([128, C], dt.float32, tag="psA")
                        nc.tensor.matmul(
                            psA[:],
                            lhsT=hT_in[:, t * 128 : (t + 1) * 128],
                            rhs=w_sb[:, :C],
                            start=True,
                            stop=True,
                        )
                        nc.vector.tensor_scalar_mul(
                            g_sb[:, t, :], psA[:], dinv_sb[:, t : t + 1]
                        )
                    nc.sync.dma_start(
                        gloc[:].rearrange("(t p) c -> p t c", p=128), g_sb[:]
                    )

                    # stage B: replicate the g table
                    if with_cc:
                        nc.gpsimd.collective_compute(
                            "AllGather",
                            OP.bypass,
                            replica_groups=RG,
                            ins=[gloc[:]],
                            outs=[gful[:]],
                        )
                    g_lo = gful[0 : min(LO_CAP, TBL), :]
                    g_hi = gful[HI_OFF:TBL, :]

                    # stage C: gather + one-hot scatter per destination tile
                    for t in range(TPC):
                        woff = t * K * 8
                        if 'nogather' not in abl:
                            msg = msg_pool.tile([128, K, C], tdt, tag="msg")
                            segs = []
                            for s0, s1, view in ((0, K_lo, g_lo), (K_lo, K, g_hi)):
                                a = s0
                                while a < s1:
                                    b = min(a + 2, s1)
                                    segs.append((a, b, view))
                                    a = b
                            for a, b, view in segs:
                                nd = (b - a) * 128
                                nc.gpsimd.dma_gather(
                                    msg[:, a:b, :],
                                    view[:, :C],
                                    idx_sb[:, woff + a * 8 : woff + b * 8],
                                    nd,
                                    nd,
                                    C,
                                    elem_step=C,
                                    single_packet=False,
                                    queue_num=next_q(nd),
                                )
                        psS = psS_pool.tile([128, C], dt.float32, tag="psS")
                        n_mm = 1 if 'nomm' in abl else K
                        if 'nooh' in abl:
                            oh_big = None
                        else:
                            # all K one-hot tiles for this dst tile in one DVE op
                            oh_big = oh_pool.tile([128, K, 128], tdt, tag="oh")
                            nc.vector.tensor_tensor(
                                oh_big[:],
                                iota_sb[:, None, :].broadcast_to([128, K, 128]),
                                dloc_sb[:, t * K : (t + 1) * K, None]
                                .broadcast_to([128, K, 128]),
                                OP.is_equal,
                            )
                        for j in range(n_mm):
                            if oh_big is None:
                                mm_lhs = ident_sb[:]
                            else:
                                mm_lhs = oh_big[:, j, :]
                            mm_rhs = (
                                msg[:, j, :]
                                if 'nogather' not in abl
                                else hT_a[:, j * 64 : j * 64 + C]
                            )
                            nc.tensor.matmul(
                                psS[:],
                                lhsT=mm_lhs,
                                rhs=mm_rhs,
                                start=(j == 0),
                                stop=(j == n_mm - 1),
                            )
                        # epilogue: + self-loop, * dinv, (+bias), relu
                        acc = eps_pool.tile([128, C], dt.float32, tag="acc")
                        nc.vector.tensor_add(acc[:], psS[:], g_sb[:, t, :])
                        h_t = eps_pool.tile([128, C], dt.float32, tag="h_t")
                        if with_bias:
                            nc.vector.tensor_scalar_mul(
                                acc[:], acc[:], dinv_sb[:, t : t + 1]
                            )
                            nc.vector.tensor_add(acc[:], acc[:], bias_sb[li][:])
                            if relu:
                                nc.scalar.activation(h_t[:], acc[:], AF.Relu)
                            else:
                                nc.scalar.copy(h_t[:], acc[:])
                        else:
                            if relu:
                                nc.scalar.activation(
                                    h_t[:], acc[:], AF.Relu,
                                    scale=dinv_sb[:, t : t + 1],
                                )
                            else:
                                nc.scalar.mul(
                                    h_t[:], acc[:], dinv_sb[:, t : t + 1]
                                )
                        if hT_out is not None:
                            psT = psT_pool.tile([128, 128], dt.float32, tag="psT")
                            nc.tensor.transpose(psT[:], h_t[:], ident_sb[:])
                            nc.vector.tensor_copy(
                                hT_out[:, t * 128 : (t + 1) * 128], psT[:]
                            )
                        else:
                            nc.sync.dma_start(
                                out_p[t * 128 : (t + 1) * 128, :], h_t[:]
                            )

            emit_layers(with_cc=True)
            if reps:
                with tc.For_i(0, reps, 1):
                    emit_layers(with_cc=False)

    nc.compile()
    return nc


_NC_CACHE: dict = {}


def _get_nc(K_lo, K_hi, with_bias):
    key = (K_lo, K_hi, with_bias)
    if key not in _NC_CACHE:
        _NC_CACHE[key] = _build_nc(K_lo, K_hi, with_bias)
    return _NC_CACHE[key]


# ---------------------------------------------------------------------------
# entry point
# ---------------------------------------------------------------------------
def _prepare(x, edge_index, W1, b1, W2, b2, W3, b3):
    import ml_dtypes

    f16 = ml_dtypes.float16 if hasattr(ml_dtypes, "float16") else np.float16
    x = np.asarray(x, dtype=np.float32)
    W1 = np.asarray(W1, dtype=np.float32)
    W2 = np.asarray(W2, dtype=np.float32)
    W3 = np.asarray(W3, dtype=np.float32)
    b1 = np.asarray(b1, dtype=np.float32)
    b2 = np.asarray(b2, dtype=np.float32)
    b3 = np.asarray(b3, dtype=np.float32)

    plan = _plan(np.asarray(edge_index))
    with_bias = bool(np.any(b1) or np.any(b2) or np.any(b3))
    nc = _get_nc(plan["K_lo"], plan["K_hi"], with_bias)

    iota = np.tile(np.arange(128, dtype=np.float32), (128, 1)).astype(f16)
    ident = np.eye(128, dtype=np.float32)
    pos_local = plan["pos_local"]

    in_maps = []
    for k in range(M_CORES):
        pc = plan["per_core"][k]
        v0 = k * NPC
        xT = np.zeros((128, NPAD), dtype=np.float32)
        xT[:, pos_local[v0 : v0 + NPC]] = x[v0 : v0 + NPC].T
        im = dict(
            xT=xT.astype(f16), W1=W1.astype(f16), W2=W2.astype(f16),
            W3=W3.astype(f16),
            dinv=pc["dinv"], dloc=pc["dloc"].astype(f16), idx=pc["idx"],
            iota=iota, ident=ident,
        )
        if with_bias:
            im["b1r"] = np.tile(b1, (128, 1)).astype(np.float32)
            im["b2r"] = np.tile(b2, (128, 1)).astype(np.float32)
            im["b3r"] = np.tile(b3, (128, 1)).astype(np.float32)
        in_maps.append(im)

    def unpermute(results):
        out = np.empty((N_NODES, OUT_C), dtype=np.float32)
        for k in range(M_CORES):
            v0 = k * NPC
            r = results[k]["out"]
            out[v0 : v0 + NPC] = r[pos_local[v0 : v0 + NPC]]
        return out

    return nc, in_maps, unpermute


def kernel(x, edge_index, W1, b1, W2, b2, W3, b3):
    from concourse.bass_utils import run_bass_kernel_spmd

    nc, in_maps, unpermute = _prepare(x, edge_index, W1, b1, W2, b2, W3, b3)
    res = run_bass_kernel_spmd(nc, in_maps, list(range(M_CORES)))
    return unpermute(res.results)


# revision 10
# speedup vs baseline: 1.1200x; 1.1200x over previous
"""3-layer GCN node classifier on 8 Trainium2 NeuronCores.

Math (per layer, folding the symmetric normalization):
    deg[v]  = in-degree(v) (with self loop), dinv = rsqrt(deg)
    g       = dinv * (h @ W)                  (rows scaled)
    out[c]  = dinv[c] * ( sum_{e: col=c} g[row_e] + g[c] ) + b
    h_next  = relu(out)      (layers 1,2; layer 3 has no relu)

Distribution: nodes are range-sharded across 8 cores (graph parallel).
Each core computes g for its own nodes (dense matmul), the g-table is
all-gathered to every core's HBM, each core then gathers the rows for
the edges whose *destination* it owns (SWDGE dma_gather) and
scatter-adds them into per-destination-tile PSUM accumulators via
one-hot matmuls on the TensorEngine.

Perf notes vs the original baseline:
  * dma_gather cost is ~per-descriptor (bytes nearly free) and a single
    SWDGE queue saturates at ~9 ns/desc; spreading calls across 4 queues
    reaches ~3 ns/desc. All gathers round-robin over queue_num 0..3.
  * layers 1-2 use an fp16 table/messages/one-hot/weights (256B gather
    rows, 2x PE rate, halved AllGather); layer 3 keeps fp32 (64ch * 2B
    = 128B would violate the 256B gather row-size constraint).
  * one-hot tiles are generated in one batched DVE op per destination
    tile (broadcast APs), fp16 output for the 2x 16-bit DVE rate.

Host-side preprocessing only touches edge_index (graph structure):
CSR-style bucketing of edges by destination tile, degree computation,
a within-core node permutation that load-balances destination tiles,
and int16 gather-index packing (the 50k-row table is split into two
overlapping <=32768-row views because SWDGE gather indices are int16).
"""

import math
import os
import numpy as np

# ---------------------------------------------------------------------------
# problem constants (hardcoded per contract; kernel.py must be self-contained)
# ---------------------------------------------------------------------------
N_NODES = 50000
IN_C, HID_C, OUT_C = 128, 128, 64
M_CORES = 8
NPC = N_NODES // M_CORES            # 6250 nodes per core
TPC = (NPC + 127) // 128            # 49 destination tiles per core
NPAD = TPC * 128                    # 6272 padded nodes per core
TBL = M_CORES * NPAD                # 50176 rows in the all-gathered table
LO_CAP = 32768                      # int16 index reach
HI_OFF = max(0, TBL - 32768)        # 17408: hi view = table[HI_OFF:]
NQ = 4                              # SWDGE queues used round-robin

F32 = "float32"


# ---------------------------------------------------------------------------
# host-side graph preprocessing (indices only)
# ---------------------------------------------------------------------------
def _plan(edge_index: np.ndarray):
    """Build per-core index/metadata arrays from edge_index [2, E]."""
    row = np.asarray(edge_index[0], dtype=np.int64)
    col = np.asarray(edge_index[1], dtype=np.int64)

    deg_in = np.bincount(col, minlength=N_NODES)          # edges only
    dinv = 1.0 / np.sqrt(deg_in + 1.0)                     # + self loop

    # within-core permutation: snake-fill tiles with degree-sorted nodes so
    # every destination tile carries a near-equal number of incoming edges.
    pos_local = np.empty(N_NODES, dtype=np.int64)
    for k in range(M_CORES):
        v0 = k * NPC
        d = deg_in[v0 : v0 + NPC]
        order = np.argsort(-d, kind="stable")              # heavy first
        # serpentine tile ids: 0..T-1, T-1..0, ...
        idx = np.arange(NPC)
        rnd, off = divmod(idx, TPC)
        tile_ids = np.where(rnd % 2 == 0, off, TPC - 1 - off)
        slot_in_tile = rnd
        pos = tile_ids * 128 + slot_in_tile
        pos_local[v0 + order] = pos

    g_pos = (np.arange(N_NODES) // NPC) * NPAD + pos_local  # table row per node

    # per-core per-tile edge buckets
    kd = col // NPC
    src_pos = g_pos[row]
    dst_slot = pos_local[col]
    dst_tile = dst_slot // 128
    dst_loc = dst_slot % 128
    is_lo = src_pos < LO_CAP

    # counts to size K_lo / K_hi uniformly across the SPMD program
    tile_key = kd * TPC + dst_tile
    n_lo = np.bincount(tile_key[is_lo], minlength=M_CORES * TPC)
    n_hi = np.bincount(tile_key[~is_lo], minlength=M_CORES * TPC)
    K_lo = max(1, int(math.ceil(n_lo.max() / 128)))
    K_hi = max(1, int(math.ceil(n_hi.max() / 128)))
    K = K_lo + K_hi
    NCH = TPC * K

    per_core = []
    for k in range(M_CORES):
        idx16 = np.zeros((TPC, K, 128), dtype=np.int16)    # pad -> row 0
        dloc_a = np.full((TPC, K, 128), 200.0, dtype=np.float32)
        m = kd == k
        tl, lc, sp, lo = dst_tile[m], dst_loc[m], src_pos[m], is_lo[m]
        for t in range(TPC):
            mt = tl == t
            for stream, base in ((lo & mt, 0), ((~lo) & mt, K_lo)):
                sps = sp[stream]
                lcs = lc[stream]
                n = sps.size
                if base == 0:
                    vals = sps
                else:
                    vals = sps - HI_OFF
                flat_i = idx16[t].reshape(-1)
                flat_d = dloc_a[t].reshape(-1)
                o = base * 128
                flat_i[o : o + n] = vals.astype(np.int16)
                flat_d[o : o + n] = lcs.astype(np.float32)

        # SWDGE wrapped-16 index layout, replicated across the 8 Q7 groups
        flat = idx16.reshape(-1)
        wrapped = flat.reshape(-1, 16).T                    # [16, NCH*8]
        idx_sb = np.tile(wrapped, (8, 1)).copy()            # [128, NCH*8]
        # per-chunk destination-local column, lane-major
        dloc_sb = dloc_a.transpose(2, 0, 1).reshape(128, NCH).copy()
        # per-slot dinv (0 on dummy slots)
        dv = np.zeros(NPAD, dtype=np.float32)
        v0 = k * NPC
        dv[pos_local[v0 : v0 + NPC]] = dinv[v0 : v0 + NPC]
        dinv_sb = dv.reshape(TPC, 128).T.copy()             # [128, TPC]
        per_core.append(dict(idx=idx_sb, dloc=dloc_sb, dinv=dinv_sb))

    return dict(
        K_lo=K_lo, K_hi=K_hi, NCH=NCH, per_core=per_core,
        pos_local=pos_local, dinv=dinv,
    )


# ---------------------------------------------------------------------------
# device program
# ---------------------------------------------------------------------------
def _build_nc(K_lo: int, K_hi: int, with_bias: bool, reps: int = 0, ablate: str = ''):
    abl = set(a for a in ablate.split(',') if a)
    """Build + compile the SPMD program.

    reps > 0 additionally emits a timing loop: the full pipeline runs once
    (correct, fills the gful tables), then a hardware For_i loop re-runs
    the whole body `reps` times with the collectives elided (collectives
    cannot sit inside control flow) so device time dominates wall clock.
    """
    import concourse.bacc as bacc
    import concourse.mybir as mybir
    from concourse import tile
    from concourse._compat import get_trn_type

    dt = mybir.dt
    K = K_lo + K_hi
    NCH = TPC * K
    NW = NCH * 8
    F16 = dt.float16

    nc = bacc.Bacc(
        get_trn_type() or "TRN2",
        target_bir_lowering=False,
        debug=False,
        enable_asserts=False,
        num_devices=M_CORES,
        num_swdge_queues=NQ,
        dynamic_dma_scratch_size=32768,
    )

    # I/O
    xT_p = nc.dram_tensor("xT", [128, NPAD], F16, kind="ExternalInput")
    W1_p = nc.dram_tensor("W1", [IN_C, HID_C], F16, kind="ExternalInput")
    W2_p = nc.dram_tensor("W2", [HID_C, HID_C], F16, kind="ExternalInput")
    W3_p = nc.dram_tensor("W3", [HID_C, OUT_C], F16, kind="ExternalInput")
    dinv_p = nc.dram_tensor("dinv", [128, TPC], dt.float32, kind="ExternalInput")
    dloc_p = nc.dram_tensor("dloc", [128, NCH], F16, kind="ExternalInput")
    idx_p = nc.dram_tensor("idx", [128, NW], dt.int16, kind="ExternalInput")
    iota_p = nc.dram_tensor("iota", [128, 128], F16, kind="ExternalInput")
    ident_p = nc.dram_tensor("ident", [128, 128], dt.float32, kind="ExternalInput")
    if with_bias:
        b1_p = nc.dram_tensor("b1r", [128, HID_C], dt.float32, kind="ExternalInput")
        b2_p = nc.dram_tensor("b2r", [128, HID_C], dt.float32, kind="ExternalInput")
        b3_p = nc.dram_tensor("b3r", [128, OUT_C], dt.float32, kind="ExternalInput")
    out_p = nc.dram_tensor("out", [NPAD, OUT_C], dt.float32, kind="ExternalOutput")

    RG = [list(range(M_CORES))]
    AF = mybir.ActivationFunctionType
    OP = mybir.AluOpType

    with tile.TileContext(nc) as tc, tc.tile_pool(name="persist", bufs=1) as pp:
        # persistent SBUF tiles (one slot each)
        hT_a = pp.tile([128, NPAD], F16, name="hT_a")
        hT_b = pp.tile([128, NPAD], F16, name="hT_b")
        w1_sb = pp.tile([128, HID_C], F16, name="w1_sb")
        w2_sb = pp.tile([128, HID_C], F16, name="w2_sb")
        w3_sb = pp.tile([128, OUT_C], F16, name="w3_sb")
        dinv_sb = pp.tile([128, TPC], dt.float32, name="dinv_sb")
        dloc_sb = pp.tile([128, NCH], F16, name="dloc_sb")
        idx_sb = pp.tile([128, NW], dt.int16, name="idx_sb")
        iota_sb = pp.tile([128, 128], F16, name="iota_sb")
        ident_sb = pp.tile([128, 128], dt.float32, name="ident_sb")
        bias_sb = []

        nc.sync.dma_start(hT_a[:], xT_p[:])
        nc.sync.dma_start(w1_sb[:], W1_p[:])
        nc.sync.dma_start(w2_sb[:], W2_p[:])
        nc.sync.dma_start(w3_sb[:], W3_p[:])
        nc.sync.dma_start(dinv_sb[:], dinv_p[:])
        nc.sync.dma_start(dloc_sb[:], dloc_p[:])
        nc.sync.dma_start(idx_sb[:], idx_p[:])
        nc.sync.dma_start(iota_sb[:], iota_p[:])
        nc.sync.dma_start(ident_sb[:], ident_p[:])
        if with_bias:
            for p, cc in ((b1_p, HID_C), (b2_p, HID_C), (b3_p, OUT_C)):
                t = pp.tile([128, cc], dt.float32, name=f"bias{len(bias_sb)}_sb")
                nc.sync.dma_start(t[:], p[:])
                bias_sb.append(t)

        # (w_sb, C, relu, table dtype, hT_in, hT_out)
        layers = [
            (w1_sb, HID_C, True, F16, hT_a, hT_b),
            (w2_sb, HID_C, True, F16, hT_b, hT_a),
            (w3_sb, OUT_C, False, dt.float32, hT_a, None),
        ]

        with (
            tc.tile_pool(name="gsb", bufs=2) as gsb_pool,
            tc.tile_pool(name="msg", bufs=2 * NQ) as msg_pool,
            tc.tile_pool(name="oh", bufs=4) as oh_pool,
            tc.tile_pool(name="eps", bufs=3) as eps_pool,
            tc.tile_pool(name="psA", bufs=2, space="PSUM") as psA_pool,
            tc.tile_pool(name="psS", bufs=2, space="PSUM") as psS_pool,
            tc.tile_pool(name="psT", bufs=2, space="PSUM") as psT_pool,
            tc.tile_pool(name="dram", bufs=1, space="DRAM") as dram_pool,
        ):
            glocs = [
                dram_pool.tile([NPAD, c], d, name=f"gloc{i}")
                for i, (c, d) in enumerate(
                    [(HID_C, F16), (HID_C, F16), (OUT_C, dt.float32)]
                )
            ]
            gfuls = [
                dram_pool.tile(
                    [TBL, c], d,
                    addr_space="Shared" if M_CORES > 4 else "Local",
                    name=f"gful{i}",
                )
                for i, (c, d) in enumerate(
                    [(HID_C, F16), (HID_C, F16), (OUT_C, dt.float32)]
                )
            ]

            qload = [0] * NQ

            def next_q(ndesc):
                q = min(range(NQ), key=lambda i: qload[i])
                qload[q] += ndesc
                return q

            def emit_layers(with_cc):
                for li, (w_sb, C, relu, tdt, hT_in, hT_out) in enumerate(layers):
                    gloc, gful = glocs[li], gfuls[li]

                    # stage A: g = dinv * (h @ W) for own nodes
                    g_sb = gsb_pool.tile([128, TPC, C], tdt, tag=f"gsb{li}")
                    for t in range(TPC):
                        psA = psA_pool.tile([128, C], dt.float32, tag="psA")
                        nc.tensor.matmul(
                            psA[:],
                            lhsT=hT_in[:, t * 128 : (t + 1) * 128],
                            rhs=w_sb[:, :C],
                            start=True,
                            stop=True,
                        )
                        nc.vector.tensor_scalar_mul(
                            g_sb[:, t, :], psA[:], dinv_sb[:, t : t + 1]
                        )
                    nc.sync.dma_start(
                        gloc[:].rearrange("(t p) c -> p t c", p=128), g_sb[:]
                    )

                    # stage B: replicate the g table
                    if with_cc:
                        nc.gpsimd.collective_compute(
                            "AllGather",
                            OP.bypass,
                            replica_groups=RG,
                            ins=[gloc[:]],
                            outs=[gful[:]],
                        )
                    g_lo = gful[0 : min(LO_CAP, TBL), :]
                    g_hi = gful[HI_OFF:TBL, :]

                    # stage C: gather + one-hot scatter per destination tile
                    for t in range(TPC):
                        woff = t * K * 8
                        if 'nogather' not in abl:
                            msg = msg_pool.tile([128, K, C], tdt, tag="msg")
                            segs = []
                            for s0, s1, view in ((0, K_lo, g_lo), (K_lo, K, g_hi)):
                                a = s0
                                while a < s1:
                                    b = min(a + 6, s1)
                                    segs.append((a, b, view))
                                    a = b
                            for a, b, view in segs:
                                nd = (b - a) * 128
                                nc.gpsimd.dma_gather(
                                    msg[:, a:b, :],
                                    view[:, :C],
                                    idx_sb[:, woff + a * 8 : woff + b * 8],
                                    nd,
                                    nd,
                                    C,
                                    elem_step=C,
                                    single_packet=False,
                                    queue_num=next_q(nd),
                                )
                        psS = psS_pool.tile([128, C], dt.float32, tag="psS")
                        n_mm = 1 if 'nomm' in abl else K
                        if 'nooh' in abl:
                            oh_big = None
                        else:
                            # all K one-hot tiles for this dst tile in one DVE op
                            oh_big = oh_pool.tile([128, K, 128], tdt, tag="oh")
                            nc.vector.tensor_tensor(
                                oh_big[:],
                                iota_sb[:, None, :].broadcast_to([128, K, 128]),
                                dloc_sb[:, t * K : (t + 1) * K, None]
                                .broadcast_to([128, K, 128]),
                                OP.is_equal,
                            )
                        for j in range(n_mm):
                            if oh_big is None:
                                mm_lhs = ident_sb[:]
                            else:
                                mm_lhs = oh_big[:, j, :]
                            mm_rhs = (
                                msg[:, j, :]
                                if 'nogather' not in abl
                                else hT_a[:, j * 64 : j * 64 + C]
                            )
                            nc.tensor.matmul(
                                psS[:],
                                lhsT=mm_lhs,
                                rhs=mm_rhs,
                                start=(j == 0),
                                stop=(j == n_mm - 1),
                            )
                        # epilogue: + self-loop, * dinv, (+bias), relu
                        acc = eps_pool.tile([128, C], dt.float32, tag="acc")
                        nc.vector.tensor_add(acc[:], psS[:], g_sb[:, t, :])
                        h_t = eps_pool.tile([128, C], dt.float32, tag="h_t")
                        if with_bias:
                            nc.vector.tensor_scalar_mul(
                                acc[:], acc[:], dinv_sb[:, t : t + 1]
                            )
                            nc.vector.tensor_add(acc[:], acc[:], bias_sb[li][:])
                            if relu:
                                nc.scalar.activation(h_t[:], acc[:], AF.Relu)
                            else:
                                nc.scalar.copy(h_t[:], acc[:])
                        else:
                            if relu:
                                nc.scalar.activation(
                                    h_t[:], acc[:], AF.Relu,
                                    scale=dinv_sb[:, t : t + 1],
                                )
                            else:
                                nc.scalar.mul(
                                    h_t[:], acc[:], dinv_sb[:, t : t + 1]
                                )
                        if hT_out is not None:
                            psT = psT_pool.tile([128, 128], dt.float32, tag="psT")
                            nc.tensor.transpose(psT[:], h_t[:], ident_sb[:])
                            nc.vector.tensor_copy(
                                hT_out[:, t * 128 : (t + 1) * 128], psT[:]
                            )
                        else:
                            nc.sync.dma_start(
                                out_p[t * 128 : (t + 1) * 128, :], h_t[:]
                            )

            emit_layers(with_cc=True)
            if reps:
                with tc.For_i(0, reps, 1):
                    emit_layers(with_cc=False)

    nc.compile()
    return nc


_NC_CACHE: dict = {}


def _get_nc(K_lo, K_hi, with_bias):
    key = (K_lo, K_hi, with_bias)
    if key not in _NC_CACHE:
        _NC_CACHE[key] = _build_nc(K_lo, K_hi, with_bias)
    return _NC_CACHE[key]


# ---------------------------------------------------------------------------
# entry point
# ---------------------------------------------------------------------------
def _prepare(x, edge_index, W1, b1, W2, b2, W3, b3):
    import ml_dtypes

    f16 = ml_dtypes.float16 if hasattr(ml_dtypes, "float16") else np.float16
    x = np.asarray(x, dtype=np.float32)
    W1 = np.asarray(W1, dtype=np.float32)
    W2 = np.asarray(W2, dtype=np.float32)
    W3 = np.asarray(W3, dtype=np.float32)
    b1 = np.asarray(b1, dtype=np.float32)
    b2 = np.asarray(b2, dtype=np.float32)
    b3 = np.asarray(b3, dtype=np.float32)

    plan = _plan(np.asarray(edge_index))
    with_bias = bool(np.any(b1) or np.any(b2) or np.any(b3))
    nc = _get_nc(plan["K_lo"], plan["K_hi"], with_bias)

    iota = np.tile(np.arange(128, dtype=np.float32), (128, 1)).astype(f16)
    ident = np.eye(128, dtype=np.float32)
    pos_local = plan["pos_local"]

    in_maps = []
    for k in range(M_CORES):
        pc = plan["per_core"][k]
        v0 = k * NPC
        xT = np.zeros((128, NPAD), dtype=np.float32)
        xT[:, pos_local[v0 : v0 + NPC]] = x[v0 : v0 + NPC].T
        im = dict(
            xT=xT.astype(f16), W1=W1.astype(f16), W2=W2.astype(f16),
            W3=W3.astype(f16),
            dinv=pc["dinv"], dloc=pc["dloc"].astype(f16), idx=pc["idx"],
            iota=iota, ident=ident,
        )
        if with_bias:
            im["b1r"] = np.tile(b1, (128, 1)).astype(np.float32)
            im["b2r"] = np.tile(b2, (128, 1)).astype(np.float32)
            im["b3r"] = np.tile(b3, (128, 1)).astype(np.float32)
        in_maps.append(im)

    def unpermute(results):
        out = np.empty((N_NODES, OUT_C), dtype=np.float32)
        for k in range(M_CORES):
            v0 = k * NPC
            r = results[k]["out"]
            out[v0 : v0 + NPC] = r[pos_local[v0 : v0 + NPC]]
        return out

    return nc, in_maps, unpermute


def kernel(x, edge_index, W1, b1, W2, b2, W3, b3):
    from concourse.bass_utils import run_bass_kernel_spmd

    nc, in_maps, unpermute = _prepare(x, edge_index, W1, b1, W2, b2, W3, b3)
    res = run_bass_kernel_spmd(nc, in_maps, list(range(M_CORES)))
    return unpermute(res.results)


# revision 11
# speedup vs baseline: 1.1683x; 1.0432x over previous
"""3-layer GCN node classifier on 8 Trainium2 NeuronCores.

Math (per layer, folding the symmetric normalization):
    deg[v]  = in-degree(v) (with self loop), dinv = rsqrt(deg)
    g       = dinv * (h @ W)                  (rows scaled)
    out[c]  = dinv[c] * ( sum_{e: col=c} g[row_e] + g[c] ) + b
    h_next  = relu(out)      (layers 1,2; layer 3 has no relu)

Distribution: nodes are range-sharded across 8 cores (graph parallel).
Each core computes g for its own nodes (dense matmul), the g-table is
all-gathered to every core's HBM, each core then gathers the rows for
the edges whose *destination* it owns (SWDGE dma_gather) and
scatter-adds them into per-destination-tile PSUM accumulators via
one-hot matmuls on the TensorEngine.

Perf notes vs the original baseline:
  * dma_gather cost is ~per-descriptor (bytes nearly free) and a single
    SWDGE queue saturates at ~9 ns/desc; spreading calls across 4 queues
    reaches ~3 ns/desc. All gathers round-robin over queue_num 0..3.
  * layers 1-2 use an fp16 table/messages/one-hot/weights (256B gather
    rows, 2x PE rate, halved AllGather); layer 3 keeps fp32 (64ch * 2B
    = 128B would violate the 256B gather row-size constraint).
  * one-hot tiles are generated in one batched DVE op per destination
    tile (broadcast APs), fp16 output for the 2x 16-bit DVE rate.

Host-side preprocessing only touches edge_index (graph structure):
CSR-style bucketing of edges by destination tile, degree computation,
a within-core node permutation that load-balances destination tiles,
and int16 gather-index packing (the 50k-row table is split into two
overlapping <=32768-row views because SWDGE gather indices are int16).
"""

import math
import os
import numpy as np

# ---------------------------------------------------------------------------
# problem constants (hardcoded per contract; kernel.py must be self-contained)
# ---------------------------------------------------------------------------
N_NODES = 50000
IN_C, HID_C, OUT_C = 128, 128, 64
M_CORES = 8
NPC = N_NODES // M_CORES            # 6250 nodes per core
TPC = (NPC + 127) // 128            # 49 destination tiles per core
NPAD = TPC * 128                    # 6272 padded nodes per core
TBL = M_CORES * NPAD                # 50176 rows in the all-gathered table
LO_CAP = 32768                      # int16 index reach
HI_OFF = max(0, TBL - 32768)        # 17408: hi view = table[HI_OFF:]
NQ = 4                              # SWDGE queues used round-robin

F32 = "float32"


# ---------------------------------------------------------------------------
# host-side graph preprocessing (indices only)
# ---------------------------------------------------------------------------
def _plan(edge_index: np.ndarray):
    """Build per-core index/metadata arrays from edge_index [2, E]."""
    row = np.asarray(edge_index[0], dtype=np.int64)
    col = np.asarray(edge_index[1], dtype=np.int64)

    deg_in = np.bincount(col, minlength=N_NODES)          # edges only
    dinv = 1.0 / np.sqrt(deg_in + 1.0)                     # + self loop

    # within-core permutation: snake-fill tiles with degree-sorted nodes so
    # every destination tile carries a near-equal number of incoming edges.
    pos_local = np.empty(N_NODES, dtype=np.int64)
    for k in range(M_CORES):
        v0 = k * NPC
        d = deg_in[v0 : v0 + NPC]
        order = np.argsort(-d, kind="stable")              # heavy first
        # serpentine tile ids: 0..T-1, T-1..0, ...
        idx = np.arange(NPC)
        rnd, off = divmod(idx, TPC)
        tile_ids = np.where(rnd % 2 == 0, off, TPC - 1 - off)
        slot_in_tile = rnd
        pos = tile_ids * 128 + slot_in_tile
        pos_local[v0 + order] = pos

    g_pos = (np.arange(N_NODES) // NPC) * NPAD + pos_local  # table row per node

    # per-core per-tile edge buckets
    kd = col // NPC
    src_pos = g_pos[row]
    dst_slot = pos_local[col]
    dst_tile = dst_slot // 128
    dst_loc = dst_slot % 128
    is_lo = src_pos < LO_CAP

    # counts to size K_lo / K_hi uniformly across the SPMD program
    tile_key = kd * TPC + dst_tile
    n_lo = np.bincount(tile_key[is_lo], minlength=M_CORES * TPC)
    n_hi = np.bincount(tile_key[~is_lo], minlength=M_CORES * TPC)
    K_lo = max(1, int(math.ceil(n_lo.max() / 128)))
    K_hi = max(1, int(math.ceil(n_hi.max() / 128)))
    K = K_lo + K_hi
    NCH = TPC * K

    per_core = []
    for k in range(M_CORES):
        idx16 = np.zeros((TPC, K, 128), dtype=np.int16)    # pad -> row 0
        dloc_a = np.full((TPC, K, 128), 200.0, dtype=np.float32)
        m = kd == k
        tl, lc, sp, lo = dst_tile[m], dst_loc[m], src_pos[m], is_lo[m]
        for t in range(TPC):
            mt = tl == t
            for stream, base in ((lo & mt, 0), ((~lo) & mt, K_lo)):
                sps = sp[stream]
                lcs = lc[stream]
                n = sps.size
                if base == 0:
                    vals = sps
                else:
                    vals = sps - HI_OFF
                flat_i = idx16[t].reshape(-1)
                flat_d = dloc_a[t].reshape(-1)
                o = base * 128
                flat_i[o : o + n] = vals.astype(np.int16)
                flat_d[o : o + n] = lcs.astype(np.float32)

        # SWDGE wrapped-16 index layout, replicated across the 8 Q7 groups
        flat = idx16.reshape(-1)
        wrapped = flat.reshape(-1, 16).T                    # [16, NCH*8]
        idx_sb = np.tile(wrapped, (8, 1)).copy()            # [128, NCH*8]
        # per-chunk destination-local column, lane-major
        dloc_sb = dloc_a.transpose(2, 0, 1).reshape(128, NCH).copy()
        # per-slot dinv (0 on dummy slots)
        dv = np.zeros(NPAD, dtype=np.float32)
        v0 = k * NPC
        dv[pos_local[v0 : v0 + NPC]] = dinv[v0 : v0 + NPC]
        dinv_sb = dv.reshape(TPC, 128).T.copy()             # [128, TPC]
        per_core.append(dict(idx=idx_sb, dloc=dloc_sb, dinv=dinv_sb))

    return dict(
        K_lo=K_lo, K_hi=K_hi, NCH=NCH, per_core=per_core,
        pos_local=pos_local, dinv=dinv,
    )


# ---------------------------------------------------------------------------
# device program
# ---------------------------------------------------------------------------
def _build_nc(K_lo: int, K_hi: int, with_bias: bool, reps: int = 0, ablate: str = ''):
    abl = set(a for a in ablate.split(',') if a)
    """Build + compile the SPMD program.

    reps > 0 additionally emits a timing loop: the full pipeline runs once
    (correct, fills the gful tables), then a hardware For_i loop re-runs
    the whole body `reps` times with the collectives elided (collectives
    cannot sit inside control flow) so device time dominates wall clock.
    """
    import concourse.bacc as bacc
    import concourse.mybir as mybir
    from concourse import tile
    from concourse._compat import get_trn_type

    dt = mybir.dt
    K = K_lo + K_hi
    NCH = TPC * K
    NW = NCH * 8
    F16 = dt.float16

    nc = bacc.Bacc(
        get_trn_type() or "TRN2",
        target_bir_lowering=False,
        debug=False,
        enable_asserts=False,
        num_devices=M_CORES,
        num_swdge_queues=NQ,
        dynamic_dma_scratch_size=32768,
    )

    # I/O
    xT_p = nc.dram_tensor("xT", [128, NPAD], F16, kind="ExternalInput")
    W1_p = nc.dram_tensor("W1", [IN_C, HID_C], F16, kind="ExternalInput")
    W2_p = nc.dram_tensor("W2", [HID_C, HID_C], F16, kind="ExternalInput")
    W3_p = nc.dram_tensor("W3", [HID_C, OUT_C], F16, kind="ExternalInput")
    dinv_p = nc.dram_tensor("dinv", [128, TPC], dt.float32, kind="ExternalInput")
    dloc_p = nc.dram_tensor("dloc", [128, NCH], F16, kind="ExternalInput")
    idx_p = nc.dram_tensor("idx", [128, NW], dt.int16, kind="ExternalInput")
    iota_p = nc.dram_tensor("iota", [128, 128], F16, kind="ExternalInput")
    ident_p = nc.dram_tensor("ident", [128, 128], dt.float32, kind="ExternalInput")
    if with_bias:
        b1_p = nc.dram_tensor("b1r", [128, HID_C], dt.float32, kind="ExternalInput")
        b2_p = nc.dram_tensor("b2r", [128, HID_C], dt.float32, kind="ExternalInput")
        b3_p = nc.dram_tensor("b3r", [128, OUT_C], dt.float32, kind="ExternalInput")
    out_p = nc.dram_tensor("out", [NPAD, OUT_C], dt.float32, kind="ExternalOutput")

    RG = [list(range(M_CORES))]
    AF = mybir.ActivationFunctionType
    OP = mybir.AluOpType

    with tile.TileContext(nc) as tc, tc.tile_pool(name="persist", bufs=1) as pp:
        # persistent SBUF tiles (one slot each)
        hT_a = pp.tile([128, NPAD], F16, name="hT_a")
        hT_b = pp.tile([128, NPAD], F16, name="hT_b")
        w1_sb = pp.tile([128, HID_C], F16, name="w1_sb")
        w2_sb = pp.tile([128, HID_C], F16, name="w2_sb")
        w3_sb = pp.tile([128, OUT_C], F16, name="w3_sb")
        dinv_sb = pp.tile([128, TPC], dt.float32, name="dinv_sb")
        dloc_sb = pp.tile([128, NCH], F16, name="dloc_sb")
        idx_sb = pp.tile([128, NW], dt.int16, name="idx_sb")
        iota_sb = pp.tile([128, 128], F16, name="iota_sb")
        ident_sb = pp.tile([128, 128], dt.float32, name="ident_sb")
        bias_sb = []

        nc.sync.dma_start(hT_a[:], xT_p[:])
        nc.sync.dma_start(w1_sb[:], W1_p[:])
        nc.sync.dma_start(w2_sb[:], W2_p[:])
        nc.sync.dma_start(w3_sb[:], W3_p[:])
        nc.sync.dma_start(dinv_sb[:], dinv_p[:])
        nc.sync.dma_start(dloc_sb[:], dloc_p[:])
        nc.sync.dma_start(idx_sb[:], idx_p[:])
        nc.sync.dma_start(iota_sb[:], iota_p[:])
        nc.sync.dma_start(ident_sb[:], ident_p[:])
        if with_bias:
            for p, cc in ((b1_p, HID_C), (b2_p, HID_C), (b3_p, OUT_C)):
                t = pp.tile([128, cc], dt.float32, name=f"bias{len(bias_sb)}_sb")
                nc.sync.dma_start(t[:], p[:])
                bias_sb.append(t)

        # (w_sb, C, relu, table dtype, hT_in, hT_out)
        layers = [
            (w1_sb, HID_C, True, F16, hT_a, hT_b),
            (w2_sb, HID_C, True, F16, hT_b, hT_a),
            (w3_sb, OUT_C, False, dt.float32, hT_a, None),
        ]

        with (
            tc.tile_pool(name="gsb", bufs=2) as gsb_pool,
            tc.tile_pool(name="msg", bufs=2 * NQ) as msg_pool,
            tc.tile_pool(name="oh", bufs=4) as oh_pool,
            tc.tile_pool(name="eps", bufs=3) as eps_pool,
            tc.tile_pool(name="psA", bufs=2, space="PSUM") as psA_pool,
            tc.tile_pool(name="psS", bufs=2, space="PSUM") as psS_pool,
            tc.tile_pool(name="psT", bufs=2, space="PSUM") as psT_pool,
            tc.tile_pool(name="dram", bufs=1, space="DRAM") as dram_pool,
        ):
            glocs = [
                dram_pool.tile([NPAD, c], d, name=f"gloc{i}")
                for i, (c, d) in enumerate(
                    [(HID_C, F16), (HID_C, F16), (OUT_C, dt.float32)]
                )
            ]
            gfuls = [
                dram_pool.tile(
                    [TBL, c], d,
                    addr_space="Shared" if M_CORES > 4 else "Local",
                    name=f"gful{i}",
                )
                for i, (c, d) in enumerate(
                    [(HID_C, F16), (HID_C, F16), (OUT_C, dt.float32)]
                )
            ]

            qload = [0] * NQ

            def next_q(ndesc):
                q = min(range(NQ), key=lambda i: qload[i])
                qload[q] += ndesc
                return q

            def emit_layers(with_cc):
                for li, (w_sb, C, relu, tdt, hT_in, hT_out) in enumerate(layers):
                    gloc, gful = glocs[li], gfuls[li]

                    # stage A: g = dinv * (h @ W) for own nodes
                    g_sb = gsb_pool.tile([128, TPC, C], tdt, tag=f"gsb{li}")
                    for t in range(TPC):
                        psA = psA_pool.tile([128, C], dt.float32, tag="psA")
                        nc.tensor.matmul(
                            psA[:],
                            lhsT=hT_in[:, t * 128 : (t + 1) * 128],
                            rhs=w_sb[:, :C],
                            start=True,
                            stop=True,
                        )
                        nc.vector.tensor_scalar_mul(
                            g_sb[:, t, :], psA[:], dinv_sb[:, t : t + 1]
                        )
                    nc.sync.dma_start(
                        gloc[:].rearrange("(t p) c -> p t c", p=128), g_sb[:]
                    )

                    # stage B: replicate the g table
                    if with_cc:
                        nc.gpsimd.collective_compute(
                            "AllGather",
                            OP.bypass,
                            replica_groups=RG,
                            ins=[gloc[:]],
                            outs=[gful[:]],
                        )
                    g_lo = gful[0 : min(LO_CAP, TBL), :]
                    g_hi = gful[HI_OFF:TBL, :]

                    # stage C: gather + one-hot scatter per destination tile
                    for t in range(TPC):
                        woff = t * K * 8
                        if 'nogather' not in abl:
                            msg = msg_pool.tile([128, K, C], tdt, tag="msg")
                            segs = []
                            for s0, s1, view in ((0, K_lo, g_lo), (K_lo, K, g_hi)):
                                a = s0
                                while a < s1:
                                    b = min(a + 4, s1)
                                    segs.append((a, b, view))
                                    a = b
                            for a, b, view in segs:
                                nd = (b - a) * 128
                                nc.gpsimd.dma_gather(
                                    msg[:, a:b, :],
                                    view[:, :C],
                                    idx_sb[:, woff + a * 8 : woff + b * 8],
                                    nd,
                                    nd,
                                    C,
                                    elem_step=C,
                                    single_packet=False,
                                    queue_num=next_q(nd),
                                )
                        psS = psS_pool.tile([128, C], dt.float32, tag="psS")
                        n_mm = 1 if 'nomm' in abl else K
                        if 'nooh' in abl:
                            oh_big = None
                        else:
                            # all K one-hot tiles for this dst tile in one DVE op
                            oh_big = oh_pool.tile([128, K, 128], tdt, tag="oh")
                            nc.vector.tensor_tensor(
                                oh_big[:],
                                iota_sb[:, None, :].broadcast_to([128, K, 128]),
                                dloc_sb[:, t * K : (t + 1) * K, None]
                                .broadcast_to([128, K, 128]),
                                OP.is_equal,
                            )
                        for j in range(n_mm):
                            if oh_big is None:
                                mm_lhs = ident_sb[:]
                            else:
                                mm_lhs = oh_big[:, j, :]
                            mm_rhs = (
                                msg[:, j, :]
                                if 'nogather' not in abl
                                else hT_a[:, j * 64 : j * 64 + C]
                            )
                            nc.tensor.matmul(
                                psS[:],
                                lhsT=mm_lhs,
                                rhs=mm_rhs,
                                start=(j == 0),
                                stop=(j == n_mm - 1),
                            )
                        # epilogue: + self-loop, * dinv, (+bias), relu
                        acc = eps_pool.tile([128, C], dt.float32, tag="acc")
                        nc.vector.tensor_add(acc[:], psS[:], g_sb[:, t, :])
                        h_t = eps_pool.tile([128, C], dt.float32, tag="h_t")
                        if with_bias:
                            nc.vector.tensor_scalar_mul(
                                acc[:], acc[:], dinv_sb[:, t : t + 1]
                            )
                            nc.vector.tensor_add(acc[:], acc[:], bias_sb[li][:])
                            if relu:
                                nc.scalar.activation(h_t[:], acc[:], AF.Relu)
                            else:
                                nc.scalar.copy(h_t[:], acc[:])
                        else:
                            if relu:
                                nc.scalar.activation(
                                    h_t[:], acc[:], AF.Relu,
                                    scale=dinv_sb[:, t : t + 1],
                                )
                            else:
                                nc.scalar.mul(
                                    h_t[:], acc[:], dinv_sb[:, t : t + 1]
                                )
                        if hT_out is not None:
                            psT = psT_pool.tile([128, 128], dt.float32, tag="psT")
                            nc.tensor.transpose(psT[:], h_t[:], ident_sb[:])
                            nc.vector.tensor_copy(
                                hT_out[:, t * 128 : (t + 1) * 128], psT[:]
                            )
                        else:
                            nc.sync.dma_start(
                                out_p[t * 128 : (t + 1) * 128, :], h_t[:]
                            )

            emit_layers(with_cc=True)
            if reps:
                with tc.For_i(0, reps, 1):
                    emit_layers(with_cc=False)

    nc.compile()
    return nc


_NC_CACHE: dict = {}


def _get_nc(K_lo, K_hi, with_bias):
    key = (K_lo, K_hi, with_bias)
    if key not in _NC_CACHE:
        _NC_CACHE[key] = _build_nc(K_lo, K_hi, with_bias)
    return _NC_CACHE[key]


# ---------------------------------------------------------------------------
# entry point
# ---------------------------------------------------------------------------
def _prepare(x, edge_index, W1, b1, W2, b2, W3, b3):
    import ml_dtypes

    f16 = ml_dtypes.float16 if hasattr(ml_dtypes, "float16") else np.float16
    x = np.asarray(x, dtype=np.float32)
    W1 = np.asarray(W1, dtype=np.float32)
    W2 = np.asarray(W2, dtype=np.float32)
    W3 = np.asarray(W3, dtype=np.float32)
    b1 = np.asarray(b1, dtype=np.float32)
    b2 = np.asarray(b2, dtype=np.float32)
    b3 = np.asarray(b3, dtype=np.float32)

    plan = _plan(np.asarray(edge_index))
    with_bias = bool(np.any(b1) or np.any(b2) or np.any(b3))
    nc = _get_nc(plan["K_lo"], plan["K_hi"], with_bias)

    iota = np.tile(np.arange(128, dtype=np.float32), (128, 1)).astype(f16)
    ident = np.eye(128, dtype=np.float32)
    pos_local = plan["pos_local"]

    in_maps = []
    for k in range(M_CORES):
        pc = plan["per_core"][k]
        v0 = k * NPC
        xT = np.zeros((128, NPAD), dtype=np.float32)
        xT[:, pos_local[v0 : v0 + NPC]] = x[v0 : v0 + NPC].T
        im = dict(
            xT=xT.astype(f16), W1=W1.astype(f16), W2=W2.astype(f16),
            W3=W3.astype(f16),
            dinv=pc["dinv"], dloc=pc["dloc"].astype(f16), idx=pc["idx"],
            iota=iota, ident=ident,
        )
        if with_bias:
            im["b1r"] = np.tile(b1, (128, 1)).astype(np.float32)
            im["b2r"] = np.tile(b2, (128, 1)).astype(np.float32)
            im["b3r"] = np.tile(b3, (128, 1)).astype(np.float32)
        in_maps.append(im)

    def unpermute(results):
        out = np.empty((N_NODES, OUT_C), dtype=np.float32)
        for k in range(M_CORES):
            v0 = k * NPC
            r = results[k]["out"]
            out[v0 : v0 + NPC] = r[pos_local[v0 : v0 + NPC]]
        return out

    return nc, in_maps, unpermute


def kernel(x, edge_index, W1, b1, W2, b2, W3, b3):
    from concourse.bass_utils import run_bass_kernel_spmd

    nc, in_maps, unpermute = _prepare(x, edge_index, W1, b1, W2, b2, W3, b3)
    res = run_bass_kernel_spmd(nc, in_maps, list(range(M_CORES)))
    return unpermute(res.results)


# revision 12
# speedup vs baseline: 1.2231x; 1.0469x over previous
"""3-layer GCN node classifier on 8 Trainium2 NeuronCores.

Math (per layer, folding the symmetric normalization):
    deg[v]  = in-degree(v) (with self loop), dinv = rsqrt(deg)
    g       = dinv * (h @ W)                  (rows scaled)
    out[c]  = dinv[c] * ( sum_{e: col=c} g[row_e] + g[c] ) + b
    h_next  = relu(out)      (layers 1,2; layer 3 has no relu)

Distribution: nodes are range-sharded across 8 cores (graph parallel).
Each core computes g for its own nodes (dense matmul), the g-table is
all-gathered to every core's HBM, each core then gathers the rows for
the edges whose *destination* it owns (SWDGE dma_gather) and
scatter-adds them into per-destination-tile PSUM accumulators via
one-hot matmuls on the TensorEngine.

Perf notes vs the original baseline:
  * dma_gather cost is ~per-descriptor (bytes nearly free) and a single
    SWDGE queue saturates at ~9 ns/desc; spreading calls across 4 queues
    reaches ~3 ns/desc. All gathers round-robin over queue_num 0..3.
  * layers 1-2 use an fp16 table/messages/one-hot/weights (256B gather
    rows, 2x PE rate, halved AllGather); layer 3 keeps fp32 (64ch * 2B
    = 128B would violate the 256B gather row-size constraint).
  * one-hot tiles are generated in one batched DVE op per destination
    tile (broadcast APs), fp16 output for the 2x 16-bit DVE rate.

Host-side preprocessing only touches edge_index (graph structure):
CSR-style bucketing of edges by destination tile, degree computation,
a within-core node permutation that load-balances destination tiles,
and int16 gather-index packing (the 50k-row table is split into two
overlapping <=32768-row views because SWDGE gather indices are int16).
"""

import math
import os
import numpy as np

# ---------------------------------------------------------------------------
# problem constants (hardcoded per contract; kernel.py must be self-contained)
# ---------------------------------------------------------------------------
N_NODES = 50000
IN_C, HID_C, OUT_C = 128, 128, 64
M_CORES = 8
NPC = N_NODES // M_CORES            # 6250 nodes per core
TPC = (NPC + 127) // 128            # 49 destination tiles per core
NPAD = TPC * 128                    # 6272 padded nodes per core
TBL = M_CORES * NPAD                # 50176 rows in the all-gathered table
LO_CAP = 32768                      # int16 index reach
HI_OFF = max(0, TBL - 32768)        # 17408: hi view = table[HI_OFF:]
NQ = 4                              # SWDGE queues used round-robin

F32 = "float32"


# ---------------------------------------------------------------------------
# host-side graph preprocessing (indices only)
# ---------------------------------------------------------------------------
def _plan(edge_index: np.ndarray):
    """Build per-core index/metadata arrays from edge_index [2, E]."""
    row = np.asarray(edge_index[0], dtype=np.int64)
    col = np.asarray(edge_index[1], dtype=np.int64)

    deg_in = np.bincount(col, minlength=N_NODES)          # edges only
    dinv = 1.0 / np.sqrt(deg_in + 1.0)                     # + self loop

    # within-core permutation: snake-fill tiles with degree-sorted nodes so
    # every destination tile carries a near-equal number of incoming edges.
    pos_local = np.empty(N_NODES, dtype=np.int64)
    for k in range(M_CORES):
        v0 = k * NPC
        d = deg_in[v0 : v0 + NPC]
        order = np.argsort(-d, kind="stable")              # heavy first
        # serpentine tile ids: 0..T-1, T-1..0, ...
        idx = np.arange(NPC)
        rnd, off = divmod(idx, TPC)
        tile_ids = np.where(rnd % 2 == 0, off, TPC - 1 - off)
        slot_in_tile = rnd
        pos = tile_ids * 128 + slot_in_tile
        pos_local[v0 + order] = pos

    g_pos = (np.arange(N_NODES) // NPC) * NPAD + pos_local  # table row per node

    # per-core per-tile edge buckets
    kd = col // NPC
    src_pos = g_pos[row]
    dst_slot = pos_local[col]
    dst_tile = dst_slot // 128
    dst_loc = dst_slot % 128
    is_lo = src_pos < LO_CAP

    # counts to size K_lo / K_hi uniformly across the SPMD program
    tile_key = kd * TPC + dst_tile
    n_lo = np.bincount(tile_key[is_lo], minlength=M_CORES * TPC)
    n_hi = np.bincount(tile_key[~is_lo], minlength=M_CORES * TPC)
    K_lo = max(1, int(math.ceil(n_lo.max() / 128)))
    K_hi = max(1, int(math.ceil(n_hi.max() / 128)))
    K = K_lo + K_hi
    NCH = TPC * K

    per_core = []
    for k in range(M_CORES):
        idx16 = np.zeros((TPC, K, 128), dtype=np.int16)    # pad -> row 0
        dloc_a = np.full((TPC, K, 128), 200.0, dtype=np.float32)
        m = kd == k
        tl, lc, sp, lo = dst_tile[m], dst_loc[m], src_pos[m], is_lo[m]
        for t in range(TPC):
            mt = tl == t
            for stream, base in ((lo & mt, 0), ((~lo) & mt, K_lo)):
                sps = sp[stream]
                lcs = lc[stream]
                n = sps.size
                if base == 0:
                    vals = sps
                else:
                    vals = sps - HI_OFF
                flat_i = idx16[t].reshape(-1)
                flat_d = dloc_a[t].reshape(-1)
                o = base * 128
                flat_i[o : o + n] = vals.astype(np.int16)
                flat_d[o : o + n] = lcs.astype(np.float32)

        # SWDGE wrapped-16 index layout, replicated across the 8 Q7 groups
        flat = idx16.reshape(-1)
        wrapped = flat.reshape(-1, 16).T                    # [16, NCH*8]
        idx_sb = np.tile(wrapped, (8, 1)).copy()            # [128, NCH*8]
        # per-chunk destination-local column, lane-major
        dloc_sb = dloc_a.transpose(2, 0, 1).reshape(128, NCH).copy()
        # per-slot dinv (0 on dummy slots)
        dv = np.zeros(NPAD, dtype=np.float32)
        v0 = k * NPC
        dv[pos_local[v0 : v0 + NPC]] = dinv[v0 : v0 + NPC]
        dinv_sb = dv.reshape(TPC, 128).T.copy()             # [128, TPC]
        per_core.append(dict(idx=idx_sb, dloc=dloc_sb, dinv=dinv_sb))

    return dict(
        K_lo=K_lo, K_hi=K_hi, NCH=NCH, per_core=per_core,
        pos_local=pos_local, dinv=dinv,
    )


# ---------------------------------------------------------------------------
# device program
# ---------------------------------------------------------------------------
def _build_nc(K_lo: int, K_hi: int, with_bias: bool, reps: int = 0, ablate: str = ''):
    abl = set(a for a in ablate.split(',') if a)
    """Build + compile the SPMD program.

    reps > 0 additionally emits a timing loop: the full pipeline runs once
    (correct, fills the gful tables), then a hardware For_i loop re-runs
    the whole body `reps` times with the collectives elided (collectives
    cannot sit inside control flow) so device time dominates wall clock.
    """
    import concourse.bacc as bacc
    import concourse.mybir as mybir
    from concourse import tile
    from concourse._compat import get_trn_type

    dt = mybir.dt
    K = K_lo + K_hi
    NCH = TPC * K
    NW = NCH * 8
    F16 = dt.float16

    nc = bacc.Bacc(
        get_trn_type() or "TRN2",
        target_bir_lowering=False,
        debug=False,
        enable_asserts=False,
        num_devices=M_CORES,
        num_swdge_queues=NQ,
    )

    # I/O
    xT_p = nc.dram_tensor("xT", [128, NPAD], F16, kind="ExternalInput")
    W1_p = nc.dram_tensor("W1", [IN_C, HID_C], F16, kind="ExternalInput")
    W2_p = nc.dram_tensor("W2", [HID_C, HID_C], F16, kind="ExternalInput")
    W3_p = nc.dram_tensor("W3", [HID_C, OUT_C], F16, kind="ExternalInput")
    dinv_p = nc.dram_tensor("dinv", [128, TPC], dt.float32, kind="ExternalInput")
    dloc_p = nc.dram_tensor("dloc", [128, NCH], F16, kind="ExternalInput")
    idx_p = nc.dram_tensor("idx", [128, NW], dt.int16, kind="ExternalInput")
    iota_p = nc.dram_tensor("iota", [128, 128], F16, kind="ExternalInput")
    ident_p = nc.dram_tensor("ident", [128, 128], dt.float32, kind="ExternalInput")
    if with_bias:
        b1_p = nc.dram_tensor("b1r", [128, HID_C], dt.float32, kind="ExternalInput")
        b2_p = nc.dram_tensor("b2r", [128, HID_C], dt.float32, kind="ExternalInput")
        b3_p = nc.dram_tensor("b3r", [128, OUT_C], dt.float32, kind="ExternalInput")
    out_p = nc.dram_tensor("out", [NPAD, OUT_C], dt.float32, kind="ExternalOutput")

    RG = [list(range(M_CORES))]
    AF = mybir.ActivationFunctionType
    OP = mybir.AluOpType

    with tile.TileContext(nc) as tc, tc.tile_pool(name="persist", bufs=1) as pp:
        # persistent SBUF tiles (one slot each)
        hT_a = pp.tile([128, NPAD], F16, name="hT_a")
        hT_b = pp.tile([128, NPAD], F16, name="hT_b")
        w1_sb = pp.tile([128, HID_C], F16, name="w1_sb")
        w2_sb = pp.tile([128, HID_C], F16, name="w2_sb")
        w3_sb = pp.tile([128, OUT_C], F16, name="w3_sb")
        dinv_sb = pp.tile([128, TPC], dt.float32, name="dinv_sb")
        dloc_sb = pp.tile([128, NCH], F16, name="dloc_sb")
        idx_sb = pp.tile([128, NW], dt.int16, name="idx_sb")
        iota_sb = pp.tile([128, 128], F16, name="iota_sb")
        ident_sb = pp.tile([128, 128], dt.float32, name="ident_sb")
        bias_sb = []

        nc.sync.dma_start(hT_a[:], xT_p[:])
        nc.sync.dma_start(w1_sb[:], W1_p[:])
        nc.sync.dma_start(w2_sb[:], W2_p[:])
        nc.sync.dma_start(w3_sb[:], W3_p[:])
        nc.sync.dma_start(dinv_sb[:], dinv_p[:])
        nc.sync.dma_start(dloc_sb[:], dloc_p[:])
        nc.sync.dma_start(idx_sb[:], idx_p[:])
        nc.sync.dma_start(iota_sb[:], iota_p[:])
        nc.sync.dma_start(ident_sb[:], ident_p[:])
        if with_bias:
            for p, cc in ((b1_p, HID_C), (b2_p, HID_C), (b3_p, OUT_C)):
                t = pp.tile([128, cc], dt.float32, name=f"bias{len(bias_sb)}_sb")
                nc.sync.dma_start(t[:], p[:])
                bias_sb.append(t)

        # (w_sb, C, relu, table dtype, hT_in, hT_out)
        layers = [
            (w1_sb, HID_C, True, F16, hT_a, hT_b),
            (w2_sb, HID_C, True, F16, hT_b, hT_a),
            (w3_sb, OUT_C, False, dt.float32, hT_a, None),
        ]

        with (
            tc.tile_pool(name="gsb", bufs=2) as gsb_pool,
            tc.tile_pool(name="msg", bufs=2 * NQ) as msg_pool,
            tc.tile_pool(name="oh", bufs=4) as oh_pool,
            tc.tile_pool(name="eps", bufs=3) as eps_pool,
            tc.tile_pool(name="psA", bufs=2, space="PSUM") as psA_pool,
            tc.tile_pool(name="psS", bufs=2, space="PSUM") as psS_pool,
            tc.tile_pool(name="psT", bufs=2, space="PSUM") as psT_pool,
            tc.tile_pool(name="dram", bufs=1, space="DRAM") as dram_pool,
        ):
            glocs = [
                dram_pool.tile([NPAD, c], d, name=f"gloc{i}")
                for i, (c, d) in enumerate(
                    [(HID_C, F16), (HID_C, F16), (OUT_C, dt.float32)]
                )
            ]
            gfuls = [
                dram_pool.tile(
                    [TBL, c], d,
                    addr_space="Shared" if M_CORES > 4 else "Local",
                    name=f"gful{i}",
                )
                for i, (c, d) in enumerate(
                    [(HID_C, F16), (HID_C, F16), (OUT_C, dt.float32)]
                )
            ]

            qload = [0] * NQ

            def next_q(ndesc):
                q = min(range(NQ), key=lambda i: qload[i])
                qload[q] += ndesc
                return q

            def emit_layers(with_cc):
                for li, (w_sb, C, relu, tdt, hT_in, hT_out) in enumerate(layers):
                    gloc, gful = glocs[li], gfuls[li]

                    # stage A: g = dinv * (h @ W) for own nodes
                    g_sb = gsb_pool.tile([128, TPC, C], tdt, tag=f"gsb{li}")
                    for t in range(TPC):
                        psA = psA_pool.tile([128, C], dt.float32, tag="psA")
                        nc.tensor.matmul(
                            psA[:],
                            lhsT=hT_in[:, t * 128 : (t + 1) * 128],
                            rhs=w_sb[:, :C],
                            start=True,
                            stop=True,
                        )
                        nc.vector.tensor_scalar_mul(
                            g_sb[:, t, :], psA[:], dinv_sb[:, t : t + 1]
                        )
                    nc.sync.dma_start(
                        gloc[:].rearrange("(t p) c -> p t c", p=128), g_sb[:]
                    )

                    # stage B: replicate the g table
                    if with_cc:
                        nc.gpsimd.collective_compute(
                            "AllGather",
                            OP.bypass,
                            replica_groups=RG,
                            ins=[gloc[:]],
                            outs=[gful[:]],
                        )
                    g_lo = gful[0 : min(LO_CAP, TBL), :]
                    g_hi = gful[HI_OFF:TBL, :]

                    # stage C: gather + one-hot scatter per destination tile
                    for t in range(TPC):
                        woff = t * K * 8
                        if 'nogather' not in abl:
                            msg = msg_pool.tile([128, K, C], tdt, tag="msg")
                            segs = []
                            for s0, s1, view in ((0, K_lo, g_lo), (K_lo, K, g_hi)):
                                a = s0
                                while a < s1:
                                    b = min(a + 3, s1)
                                    segs.append((a, b, view))
                                    a = b
                            for a, b, view in segs:
                                nd = (b - a) * 128
                                nc.gpsimd.dma_gather(
                                    msg[:, a:b, :],
                                    view[:, :C],
                                    idx_sb[:, woff + a * 8 : woff + b * 8],
                                    nd,
                                    nd,
                                    C,
                                    elem_step=C,
                                    single_packet=False,
                                    queue_num=next_q(nd),
                                )
                        psS = psS_pool.tile([128, C], dt.float32, tag="psS")
                        n_mm = 1 if 'nomm' in abl else K
                        if 'nooh' in abl:
                            oh_big = None
                        else:
                            # all K one-hot tiles for this dst tile in one DVE op
                            oh_big = oh_pool.tile([128, K, 128], tdt, tag="oh")
                            nc.vector.tensor_tensor(
                                oh_big[:],
                                iota_sb[:, None, :].broadcast_to([128, K, 128]),
                                dloc_sb[:, t * K : (t + 1) * K, None]
                                .broadcast_to([128, K, 128]),
                                OP.is_equal,
                            )
                        for j in range(n_mm):
                            if oh_big is None:
                                mm_lhs = ident_sb[:]
                            else:
                                mm_lhs = oh_big[:, j, :]
                            mm_rhs = (
                                msg[:, j, :]
                                if 'nogather' not in abl
                                else hT_a[:, j * 64 : j * 64 + C]
                            )
                            nc.tensor.matmul(
                                psS[:],
                                lhsT=mm_lhs,
                                rhs=mm_rhs,
                                start=(j == 0),
                                stop=(j == n_mm - 1),
                            )
                        # epilogue: + self-loop, * dinv, (+bias), relu
                        acc = eps_pool.tile([128, C], dt.float32, tag="acc")
                        nc.vector.tensor_add(acc[:], psS[:], g_sb[:, t, :])
                        h_t = eps_pool.tile([128, C], dt.float32, tag="h_t")
                        if with_bias:
                            nc.vector.tensor_scalar_mul(
                                acc[:], acc[:], dinv_sb[:, t : t + 1]
                            )
                            nc.vector.tensor_add(acc[:], acc[:], bias_sb[li][:])
                            if relu:
                                nc.scalar.activation(h_t[:], acc[:], AF.Relu)
                            else:
                                nc.scalar.copy(h_t[:], acc[:])
                        else:
                            if relu:
                                nc.scalar.activation(
                                    h_t[:], acc[:], AF.Relu,
                                    scale=dinv_sb[:, t : t + 1],
                                )
                            else:
                                nc.scalar.mul(
                                    h_t[:], acc[:], dinv_sb[:, t : t + 1]
                                )
                        if hT_out is not None:
                            psT = psT_pool.tile([128, 128], dt.float32, tag="psT")
                            nc.tensor.transpose(psT[:], h_t[:], ident_sb[:])
                            nc.vector.tensor_copy(
                                hT_out[:, t * 128 : (t + 1) * 128], psT[:]
                            )
                        else:
                            nc.sync.dma_start(
                                out_p[t * 128 : (t + 1) * 128, :], h_t[:]
                            )

            emit_layers(with_cc=True)
            if reps:
                with tc.For_i(0, reps, 1):
                    emit_layers(with_cc=False)

    nc.compile()
    return nc


_NC_CACHE: dict = {}


def _get_nc(K_lo, K_hi, with_bias):
    key = (K_lo, K_hi, with_bias)
    if key not in _NC_CACHE:
        _NC_CACHE[key] = _build_nc(K_lo, K_hi, with_bias)
    return _NC_CACHE[key]


# ---------------------------------------------------------------------------
# entry point
# ---------------------------------------------------------------------------
def _prepare(x, edge_index, W1, b1, W2, b2, W3, b3):
    import ml_dtypes

    f16 = ml_dtypes.float16 if hasattr(ml_dtypes, "float16") else np.float16
    x = np.asarray(x, dtype=np.float32)
    W1 = np.asarray(W1, dtype=np.float32)
    W2 = np.asarray(W2, dtype=np.float32)
    W3 = np.asarray(W3, dtype=np.float32)
    b1 = np.asarray(b1, dtype=np.float32)
    b2 = np.asarray(b2, dtype=np.float32)
    b3 = np.asarray(b3, dtype=np.float32)

    plan = _plan(np.asarray(edge_index))
    with_bias = bool(np.any(b1) or np.any(b2) or np.any(b3))
    nc = _get_nc(plan["K_lo"], plan["K_hi"], with_bias)

    iota = np.tile(np.arange(128, dtype=np.float32), (128, 1)).astype(f16)
    ident = np.eye(128, dtype=np.float32)
    pos_local = plan["pos_local"]

    in_maps = []
    for k in range(M_CORES):
        pc = plan["per_core"][k]
        v0 = k * NPC
        xT = np.zeros((128, NPAD), dtype=np.float32)
        xT[:, pos_local[v0 : v0 + NPC]] = x[v0 : v0 + NPC].T
        im = dict(
            xT=xT.astype(f16), W1=W1.astype(f16), W2=W2.astype(f16),
            W3=W3.astype(f16),
            dinv=pc["dinv"], dloc=pc["dloc"].astype(f16), idx=pc["idx"],
            iota=iota, ident=ident,
        )
        if with_bias:
            im["b1r"] = np.tile(b1, (128, 1)).astype(np.float32)
            im["b2r"] = np.tile(b2, (128, 1)).astype(np.float32)
            im["b3r"] = np.tile(b3, (128, 1)).astype(np.float32)
        in_maps.append(im)

    def unpermute(results):
        out = np.empty((N_NODES, OUT_C), dtype=np.float32)
        for k in range(M_CORES):
            v0 = k * NPC
            r = results[k]["out"]
            out[v0 : v0 + NPC] = r[pos_local[v0 : v0 + NPC]]
        return out

    return nc, in_maps, unpermute


def kernel(x, edge_index, W1, b1, W2, b2, W3, b3):
    from concourse.bass_utils import run_bass_kernel_spmd

    nc, in_maps, unpermute = _prepare(x, edge_index, W1, b1, W2, b2, W3, b3)
    res = run_bass_kernel_spmd(nc, in_maps, list(range(M_CORES)))
    return unpermute(res.results)
